# revision 54
# baseline (speedup 1.0000x reference)
"""Trainium2 Bass kernel for nn_AttnResBlock (B=64, CH1=3, CH2=4, HID=16,
T=16384, E=512).

Strategy: tensor-parallel split of the T dimension across 8 cores, bf16
compute, with the small channel-mix matrices folded algebraically so the
per-batch attention stage is tiny.

  y = p + W3 @ (attn(W1@p, W2@c | Wq,Wk,Wv) @ Wo) + W3@bo,   p=x[:,:3], c=x[:,3:7]

Stage A (T-parallel): each core owns a 2048-wide T slice and contracts
  x^T[t,(c,b)] against Wq/Wk/Wv[t,e] slices -> partial Cq/Ck/Cv[(c,b),e].
  Partials (11 channels x 64 batches, bf16) ReduceScatter-add so core r
  receives batches 8r..8r+7.

Stage B (B-parallel): per batch, everything is folded through the tiny
  channel mixes:
    S^T[f,e] = kin_aug^T (W2a W1a^T) qin_aug   via G = M'^T-matmul (K=4)
    Et = exp(S^T * scale)                      (ACT, bf16 out)
    V3t[f,o] = vin_aug^T W32^T  (W32 = W3 W2a^T), col 3 = ones -> Z
    attn3^T[e,(o,Z)] = sum_f Et[f,e] V3t[f,o]  (16 N=4 matmuls)
  attn3/Z is normalized into the AllGather staging tile [e,(b,o)].
  Biases enter as augmented rows: data rows come from the RS; bias rows are
  loaded locally (not reduced); vin gets a const-ones row for the Z column.

Stage C (T-parallel): y^T[t,(b,o)] = sum_e Wo^T[e,t] attn3n[e,(b,o)]
  + bo[t] (x) w3sum[(b,o)] (K=1 rank-1 matmul) + residual p^T straight out
  of the already-loaded x^T tile. One fp32 output DMA per half.

All heavy tensors move as bf16 (tolerance is 2e-2; bf16 lands ~1e-3) and as
a handful of large contiguous DMAs - the cost model charges ~0.65us of
shared HWDGE/SEQ time per dma_start, which dominated the previous version.
"""

import numpy as np
import ml_dtypes

import concourse.bacc as bacc
import concourse.tile as tile
import concourse.mybir as mybir

F32 = mybir.dt.float32
BF16 = mybir.dt.bfloat16
AF = mybir.ActivationFunctionType
ALU = mybir.AluOpType

B, CH1, CH2, HID, T, E = 64, 3, 4, 16, 16384, 512
NCORES = 8
TLOC = T // NCORES          # 2048
KT = TLOC // 128            # 16 k-tiles in stage A
EC = E // 128               # 4 e/f chunks
BLOC = B // NCORES          # 8 batches per core in stage B
NCH = CH1 + CH2 + CH2       # 11 data rows per batch through the RS
XW = 7 * B                  # 448 x^T columns per k-tile
GW = CH1 * BLOC             # 24 AllGather columns per rank
SCALE = 1.0 / np.sqrt(HID)
NWARM = 29                  # PE p-state warmup matmuls (see build_rep)
PADSQ = (0, 0, 0, 0)       # PE pads between q quarter-blocks
PADRS = 0                  # PE pad across the ReduceScatter window
PADAG = 22                  # PE pad across the AllGather window


def build_program(reps: int = 1):
    nc = bacc.Bacc("TRN2", target_bir_lowering=False, debug=False,
                   num_devices=NCORES)

    xt = nc.dram_tensor("xt", [128, KT * XW], BF16, kind="ExternalInput")
    wq = nc.dram_tensor("wq", [128, KT * E], BF16, kind="ExternalInput")
    wk = nc.dram_tensor("wk", [128, KT * E], BF16, kind="ExternalInput")
    wv = nc.dram_tensor("wv", [128, KT * E], BF16, kind="ExternalInput")
    wot = nc.dram_tensor("wot", [128, EC * TLOC], BF16, kind="ExternalInput")
    bias3 = nc.dram_tensor("bias3", [4, BLOC * E], BF16, kind="ExternalInput")
    m5t = nc.dram_tensor("m5t", [CH1 + 1, CH2 + 1], BF16,
                         kind="ExternalInput")
    w32t = nc.dram_tensor("w32t", [CH2 + 2, CH1 + 1], BF16,
                          kind="ExternalInput")
    botw = nc.dram_tensor("botw", [1, TLOC + CH1 * B], BF16,
                          kind="ExternalInput")
    yt = nc.dram_tensor("yt", [128, KT * CH1 * B], BF16,
                        kind="ExternalOutput")

    rg = [list(range(NCORES))]

    with tile.TileContext(nc) as tc:
        with tc.tile_pool(name="const", bufs=1) as cp, \
             tc.tile_pool(name="dram", space="DRAM", bufs=1) as dp, \
             tc.tile_pool(name="big", bufs=1) as bp:
            m5t_s = cp.tile([CH1 + 1, CH2 + 1], BF16, name="m5t_s")
            w32t_s = cp.tile([CH2 + 2, CH1 + 1], BF16, name="w32t_s")
            botw_s = cp.tile([1, TLOC + CH1 * B], BF16, name="botw_s")
            nc.scalar.dma_start(m5t_s[:], m5t[:])
            nc.scalar.dma_start(w32t_s[:], w32t[:])
            nc.scalar.dma_start(botw_s[:], botw[:])
            for rep in range(reps):
                build_rep(nc, tc, dp, bp, cp, rep, rg, locals())

    nc.compile()
    return nc


def build_rep(nc, tc, dp, bp, cp, rep, rg, env):
    xt, wq, wk, wv, wot, yt = (env[k] for k in
                               ("xt", "wq", "wk", "wv", "wot", "yt"))
    bias3 = env["bias3"]
    m5t_s, w32t_s, botw_s = env["m5t_s"], env["w32t_s"], env["botw_s"]
    r = f"r{rep}"

    # ---- DRAM bounce buffers for the collectives -------------------------
    rs_in = dp.tile([B, NCH * E], BF16, name=f"rs_in_{r}", tag="rs_in",
                    bufs=1)
    rs_out = dp.tile([BLOC, NCH * E], BF16, name=f"rs_out_{r}", tag="rs_out",
                     bufs=1)
    ag_in = dp.tile([E, GW], BF16, name=f"ag_in_{r}", tag="ag_in", bufs=1)
    ag_out = dp.tile([NCORES * E, GW], BF16, name=f"ag_out_{r}", tag="ag_out",
                     bufs=1)

    # ---- persistent SBUF tiles ------------------------------------------
    xt_s = bp.tile([128, KT * XW], BF16, name=f"xt_{r}", tag="xt", bufs=1)
    wq_s = bp.tile([128, KT * E], BF16, name=f"wq_{r}", tag="wq", bufs=1)
    wk_s = bp.tile([128, KT * E], BF16, name=f"wk_{r}", tag="wk", bufs=1)
    wv_s = bp.tile([128, KT * E], BF16, name=f"wv_{r}", tag="wv", bufs=1)
    wot_s = bp.tile([128, EC * TLOC], BF16, name=f"wot_{r}", tag="wot",
                    bufs=1)
    qall = bp.tile([CH1 + 1, BLOC * E], BF16, name=f"qall_{r}", tag="qall",
                   bufs=1)
    kall = bp.tile([CH2 + 1, BLOC * E], BF16, name=f"kall_{r}", tag="kall",
                   bufs=1)
    vall = bp.tile([CH2 + 2, BLOC * E], BF16, name=f"vall_{r}", tag="vall",
                   bufs=1)
    sAq = bp.tile([64, CH1 * E], BF16, name=f"sAq_{r}", tag="sAq", bufs=1)
    sAk = bp.tile([64, CH2 * E], BF16, name=f"sAk_{r}", tag="sAk", bufs=1)
    sAv = bp.tile([64, CH2 * E], BF16, name=f"sAv_{r}", tag="sAv", bufs=1)
    ag_s = bp.tile([128, EC * GW], BF16, name=f"ag_s_{r}", tag="ag_s", bufs=1)
    at3_s = bp.tile([128, EC * NCORES * GW], BF16, name=f"at3_{r}", tag="at3",
                    bufs=1)
    y_all = bp.tile([128, KT * CH1 * B], BF16, name=f"y_all_{r}",
                    tag="y_all", bufs=1)

    # stage-A inputs on the sync queue in PE consumption order, chunked so
    # the PE trails the DMA stream with minimal lag
    QX, QW, HW = KT * XW // 4, KT * E // 4, KT * E // 2
    for qc in range(4):
        nc.sync.dma_start(xt_s[:, qc * QX:(qc + 1) * QX],
                          xt[:, qc * QX:(qc + 1) * QX])
        nc.sync.dma_start(wq_s[:, qc * QW:(qc + 1) * QW],
                          wq[:, qc * QW:(qc + 1) * QW])
    for qc in range(4):
        nc.sync.dma_start(wk_s[:, qc * QW:(qc + 1) * QW],
                          wk[:, qc * QW:(qc + 1) * QW])
    for qc in range(4):
        nc.sync.dma_start(wv_s[:, qc * QW:(qc + 1) * QW],
                          wv[:, qc * QW:(qc + 1) * QW])

    # stage-B constants on the scalar queue (tiny; land during stage A)
    nc.scalar.dma_start(qall[CH1:CH1 + 1, :], bias3[0:1, :])
    nc.scalar.dma_start(kall[CH2:CH2 + 1, :], bias3[1:2, :])
    nc.scalar.dma_start(vall[CH2:CH2 + 2, :], bias3[2:4, :])

    # PE p-state warmup / gap padding: the tensor engine drops to a low clock
    # after idling, and a burst dispatched right after a semaphore wait is
    # costed at the slow clock. Dummy matmuls (no data deps) keep the PE busy
    # through DMA-wait windows so real matmuls run at the ramped rate. Also
    # preload the Exp activation table off the critical path.
    dw = bp.tile([128, 256], BF16, name=f"dw_{r}", tag="dw", bufs=1)
    nc.vector.memset(dw[:], 0.0)
    dex = bp.tile([1, 2], F32, name=f"dex_{r}", tag="dex", bufs=1)
    nc.vector.memset(dex[0:1, 0:1], 0.0)
    nc.scalar.activation(dex[0:1, 1:2], dex[0:1, 0:1], AF.Exp)

    def pad_chain(psum_tile, n, cols=256):
        for i in range(n):
            nc.tensor.matmul(psum_tile[:, 0:cols], dw[:, 0:128],
                             dw[:, 0:cols], start=(i == 0), stop=(i == n - 1))

    psG_cm = tc.tile_pool(name="psG", space="PSUM", bufs=1)
    psG = psG_cm.__enter__()
    with tc.tile_pool(name="psW", space="PSUM", bufs=1) as psW:
        dps = psW.tile([128, 256], F32, name=f"dps_{r}", tag="dps", bufs=1)
        pad_chain(dps, NWARM)

    # ---- stage A: big T-contraction (bf16, PE fed in DMA arrival order) --
    with tc.tile_pool(name="psA", space="PSUM", bufs=1) as psA:
        p0 = psA.tile([128, E], F32, name=f"p0_{r}", tag="p0", bufs=1)
        p1 = psA.tile([64, E], F32, name=f"p1_{r}", tag="p1", bufs=1)
        k0 = psA.tile([128, E], F32, name=f"k0_{r}", tag="k0", bufs=1)
        k1 = psA.tile([128, E], F32, name=f"k1_{r}", tag="k1", bufs=1)
        v0 = psA.tile([128, E], F32, name=f"v0_{r}", tag="v0", bufs=1)
        v1 = psA.tile([128, E], F32, name=f"v1_{r}", tag="v1", bufs=1)

        dps2 = psA.tile([128, 256], F32, name=f"dps2_{r}", tag="dps2",
                        bufs=1)

        def xk(k, a, b):
            return xt_s[:, k * XW + a:k * XW + b]

        def copies(sA, chmap):
            # PSUM -> bf16 staging in (b, ch, e) order, split over ACT + DVE
            for i, (ps, off, ch) in enumerate(chmap):
                dst = sA[:, ch * E:(ch + 1) * E]
                src = ps[off:off + 64, :]
                if i % 2 == 0:
                    nc.scalar.activation(dst, src, AF.Copy)
                else:
                    nc.vector.tensor_copy(dst, src)

        def ablock(ws, d0, d1, klo, khi):
            for k in range(klo, khi):
                st, sp = (k == 0), (k == KT - 1)
                w = ws[:, k * E:(k + 1) * E]
                nc.tensor.matmul(d0[:], xk(k, 192 if d0 is not p0 else 0,
                                           320 if d0 is not p0 else 128),
                                 w, start=st, stop=sp)
                nc.tensor.matmul(d1[:], xk(k, 320 if d1 is not p1 else 128,
                                           448 if d1 is not p1 else 192),
                                 w, start=st, stop=sp)

        for qc in range(4):
            ablock(wq_s, p0, p1, qc * KT // 4, (qc + 1) * KT // 4)
            pad_chain(dps2, PADSQ[qc])
        copies(sAq, [(p0, 0, 0), (p0, 64, 1), (p1, 0, 2)])
        rs_v3 = rs_in[:].rearrange("b (c e) -> b c e", c=NCH)
        nc.sync.dma_start(rs_v3[:, 0:CH1, :],
                          sAq[:].rearrange("b (c e) -> b c e", c=CH1))
        for qc in range(4):
            ablock(wk_s, k0, k1, qc * KT // 4, (qc + 1) * KT // 4)
        copies(sAk, [(k0, 0, 0), (k0, 64, 1), (k1, 0, 2), (k1, 64, 3)])
        nc.sync.dma_start(rs_v3[:, CH1:CH1 + CH2, :],
                          sAk[:].rearrange("b (c e) -> b c e", c=CH2))
        for qc in range(4):
            ablock(wv_s, v0, v1, qc * KT // 4, (qc + 1) * KT // 4)
        copies(sAv, [(v0, 0, 0), (v0, 64, 1), (v1, 0, 2), (v1, 64, 3)])
        nc.sync.dma_start(rs_v3[:, CH1 + CH2:NCH, :],
                          sAv[:].rearrange("b (c e) -> b c e", c=CH2))
        # stage-C weights dispatch behind the sem-gated rs writes, so their
        # transfers ride the collective window instead of the input stream
        for wc in range(4):
            nc.sync.dma_start(wot_s[:, wc * TLOC:(wc + 1) * TLOC],
                              wot[:, wc * TLOC:(wc + 1) * TLOC])


    nc.gpsimd.collective_compute(
        "ReduceScatter", ALU.add, replica_groups=rg,
        ins=[rs_in.opt()], outs=[rs_out.opt()],
    )

    # scatter reduced rows into the partition-aligned q/k/v group tiles:
    # partitions = group row, free = (b, e)
    rs_c = rs_out[:].rearrange("b (c e) -> c b e", c=NCH)
    nc.scalar.dma_start(
        qall[0:CH1, :].rearrange("r (b e) -> r b e", b=BLOC),
        rs_c[0:CH1])
    nc.scalar.dma_start(
        kall[0:CH2, :].rearrange("r (b e) -> r b e", b=BLOC),
        rs_c[CH1:CH1 + CH2])
    nc.scalar.dma_start(
        vall[0:CH2, :].rearrange("r (b e) -> r b e", b=BLOC),
        rs_c[CH1 + CH2:NCH])


    # ---- stage B: per-batch attention, software-pipelined ----------------
    with tc.tile_pool(name="psB", space="PSUM", bufs=1) as psB, \
         tc.tile_pool(name="sbB", bufs=1) as sb:

        # 3-deep software pipeline over batches so the per-batch dependency
        # chain G(PE) -> copy(DVE) -> S(PE) -> exp(ACT) -> attn(PE) ->
        # norm(DVE) is hidden: in iteration i the PE runs G(i), V3/S(i-1),
        # attn(i-2), the ACT exps batch i-1, the DVE finishes batch i-2.
        st_g, st_m = {}, {}

        def fstage(b):
            qin = qall[:, b * E:(b + 1) * E]
            g_ps = psG.tile([CH2 + 1, E], F32, name=f"gps{b}_{r}", tag="gps",
                            bufs=1)
            nc.tensor.matmul(g_ps[:], m5t_s[:], qin, start=True, stop=True)
            g_s = sb.tile([CH2 + 1, E], BF16, name=f"gs{b}_{r}", tag="gs",
                          bufs=2)
            nc.vector.tensor_copy(g_s[:], g_ps[:])
            st_g[b] = g_s

        def mstage(b):
            g_s = st_g.pop(b)
            kin = kall[:, b * E:(b + 1) * E]
            vin = vall[:, b * E:(b + 1) * E]
            v3_ps = psB.tile([128, EC * (CH1 + 1)], F32, name=f"v3ps{b}_{r}",
                             tag="v3ps", bufs=1)
            for fc in range(EC):
                nc.tensor.matmul(v3_ps[:, fc * 4:(fc + 1) * 4],
                                 vin[:, fc * 128:(fc + 1) * 128],
                                 w32t_s[:], start=True, stop=True)
            v3_s = sb.tile([128, EC * (CH1 + 1)], BF16, name=f"v3s{b}_{r}",
                           tag="v3s", bufs=2)
            nc.vector.tensor_copy(v3_s[:], v3_ps[:])
            es = sb.tile([128, EC * E], BF16, name=f"es{b}_{r}", tag="es",
                         bufs=2)
            for h in range(2):
                s_ps = psB.tile([128, 2 * E], F32, name=f"sps{b}{h}_{r}",
                                tag="sps", bufs=2)
                for j in range(2):
                    fc = h * 2 + j
                    nc.tensor.matmul(s_ps[:, j * E:(j + 1) * E],
                                     kin[:, fc * 128:(fc + 1) * 128],
                                     g_s[:], start=True, stop=True)
                nc.scalar.activation(es[:, h * 2 * E:(h + 1) * 2 * E],
                                     s_ps[:], AF.Exp, scale=SCALE)
            st_m[b] = (es, v3_s)

        def bstage(b):
            es, v3_s = st_m.pop(b)
            a3_ps = psB.tile([128, EC * (CH1 + 1)], F32, name=f"a3ps{b}_{r}",
                             tag="a3ps", bufs=1)
            for ec in range(EC):
                for fc in range(EC):
                    nc.tensor.matmul(
                        a3_ps[:, ec * 4:(ec + 1) * 4],
                        es[:, fc * E + ec * 128:fc * E + (ec + 1) * 128],
                        v3_s[:, fc * 4:(fc + 1) * 4],
                        start=(fc == 0), stop=(fc == EC - 1))
            zr = sb.tile([128, EC], F32, name=f"zr{b}_{r}", tag="zr", bufs=2)
            a3_v = a3_ps[:].rearrange("p (ec k) -> p ec k", k=CH1 + 1)
            nc.vector.reciprocal(zr[:], a3_v[:, :, CH1])
            for ec in range(EC):
                nc.vector.tensor_scalar(
                    out=ag_s[:, ec * GW + b * CH1:ec * GW + b * CH1 + CH1],
                    in0=a3_ps[:, ec * 4:ec * 4 + CH1],
                    scalar1=zr[:, ec:ec + 1], scalar2=None, op0=ALU.mult)

        for i in range(BLOC + 2):
            if i < BLOC:
                fstage(i)
            if 0 <= i - 1 < BLOC:
                mstage(i - 1)
            if i - 2 >= 0:
                bstage(i - 2)

        nc.sync.dma_start(
            ag_in[:].rearrange("(ec p) c -> p ec c", p=128),
            ag_s[:].rearrange("p (ec c) -> p ec c", c=GW))
        # keep the PE clock ramped across the AllGather window (reuses an
        # sps PSUM buffer; waits for its last exp read, then free-runs)
        dps4 = psB.tile([128, 2 * E], F32, name=f"dps4_{r}", tag="sps",
                        bufs=2)
        pad_chain(dps4, PADAG)

    psG_cm.__exit__(None, None, None)

    nc.gpsimd.collective_compute(
        "AllGather", ALU.bypass, replica_groups=rg,
        ins=[ag_in.opt()], outs=[ag_out.opt()],
    )

    # ---- stage C: y^T[t,(g,bl,o)] = Wo^T-contraction + bias + residual ---
    ag_v = ag_out[:].rearrange("(g ec p) c -> ec p g c", g=NCORES, ec=EC)
    for ec in range(EC):
        eng = nc.sync if ec % 2 == 0 else nc.scalar
        eng.dma_start(
            at3_s[:, ec * NCORES * GW:(ec + 1) * NCORES * GW].rearrange(
                "p (g c) -> p g c", g=NCORES),
            ag_v[ec])

    xres = xt_s[:].rearrange("p (k c g bl) -> p k g bl c",
                             k=KT, c=7, g=NCORES)
    with tc.tile_pool(name="psC", space="PSUM", bufs=1) as psC:
        for t in range(KT):
            y_ps = psC.tile([128, CH1 * B], F32, name=f"yps{t}_{r}",
                            tag="yps", bufs=3)
            for ec in range(EC):
                nc.tensor.matmul(
                    y_ps[:],
                    wot_s[:, ec * TLOC + t * 128:ec * TLOC + (t + 1) * 128],
                    at3_s[:, ec * NCORES * GW:(ec + 1) * NCORES * GW],
                    start=(ec == 0), stop=False)
            nc.tensor.matmul(y_ps[:], botw_s[:, t * 128:(t + 1) * 128],
                             botw_s[:, TLOC:TLOC + CH1 * B],
                             start=False, stop=True)
            dst = y_all[:, t * CH1 * B:(t + 1) * CH1 * B].rearrange(
                "p (g bl c) -> p g bl c", g=NCORES, bl=BLOC)
            src = y_ps[:].rearrange("p (g bl c) -> p g bl c",
                                    g=NCORES, bl=BLOC)
            res = xres[:, t, :, :, 0:CH1]
            nc.vector.tensor_tensor(out=dst, in0=src, in1=res, op=ALU.add)
            if t % 4 == 3:
                c0, c1 = (t - 3) * CH1 * B, (t + 1) * CH1 * B
                nc.sync.dma_start(yt[:, c0:c1], y_all[:, c0:c1])


_CACHE = {}


def _get_program(reps: int):
    if reps not in _CACHE:
        _CACHE[reps] = build_program(reps)
    return _CACHE[reps]


class _PjrtRunner:
    """jit-once wrapper around bass2jax so repeat calls skip recompile/reload.

    Mirrors bass2jax.run_bass_via_pjrt's multi-core shard_map path, but keeps
    the jitted callable alive so the NEFF stays loaded on the devices and
    repeat invocations measure execution (+ host transfer) only.
    """

    def __init__(self, nc):
        import jax
        from jax.sharding import Mesh, PartitionSpec
        from jax.experimental.shard_map import shard_map
        from concourse import bass2jax

        bass2jax.install_neuronx_cc_hook()
        self.nc = nc
        in_names, out_names, out_avals, zero_outs = [], [], [], []
        partition_name = (nc.partition_id_tensor.name
                          if nc.partition_id_tensor else None)
        for alloc in nc.m.functions[0].allocations:
            if not isinstance(alloc, mybir.MemoryLocationSet):
                continue
            name = alloc.memorylocations[0].name
            if alloc.kind == "ExternalInput":
                if name != partition_name:
                    in_names.append(name)
            elif alloc.kind == "ExternalOutput":
                shape = tuple(alloc.tensor_shape)
                dtype = mybir.dt.np(alloc.dtype)
                out_names.append(name)
                out_avals.append(jax.core.ShapedArray(shape, dtype))
                zero_outs.append(np.zeros(shape, dtype))
        self.n_params = len(in_names)
        self.in_names = list(in_names)
        self.out_names = out_names
        self.out_avals = out_avals
        self.zero_outs = zero_outs
        all_in_names = in_names + out_names
        if partition_name is not None:
            all_in_names.append(partition_name)

        n_outs = len(out_names)
        donate = tuple(range(self.n_params, self.n_params + n_outs))

        def _body(*args):
            operands = list(args)
            if partition_name is not None:
                operands.append(bass2jax.partition_id_tensor())
            outs = bass2jax._bass_exec_p.bind(
                *operands,
                out_avals=tuple(out_avals),
                in_names=tuple(all_in_names),
                out_names=tuple(out_names),
                lowering_input_output_aliases=(),
                sim_require_finite=True,
                sim_require_nnan=True,
                nc=nc,
            )
            return tuple(outs)

        devices = jax.devices()[:NCORES]
        mesh = Mesh(np.asarray(devices), ("core",))
        self.mesh = mesh
        in_specs = (PartitionSpec("core"),) * (self.n_params + n_outs)
        out_specs = (PartitionSpec("core"),) * n_outs
        self.fn = jax.jit(
            shard_map(_body, mesh=mesh, in_specs=in_specs,
                      out_specs=out_specs, check_rep=False),
            donate_argnums=donate, keep_unused=True)

    def __call__(self, in_maps):
        concat_in = [
            np.concatenate([np.asarray(in_maps[c][nm]) for c in range(NCORES)],
                           axis=0)
            for nm in self.in_names]
        concat_zeros = [
            np.zeros((NCORES * z.shape[0], *z.shape[1:]), z.dtype)
            for z in self.zero_outs]
        out_arrs = self.fn(*concat_in, *concat_zeros)
        return [
            {nm: np.asarray(out_arrs[i]).reshape(
                NCORES, *self.out_avals[i].shape)[c]
             for i, nm in enumerate(self.out_names)}
            for c in range(NCORES)]


_RUNNERS = {}


def _get_runner(reps: int):
    if reps not in _RUNNERS:
        _RUNNERS[reps] = _PjrtRunner(_get_program(reps))
    return _RUNNERS[reps]


def bf(a):
    return np.ascontiguousarray(np.asarray(a, np.float32)).astype(
        ml_dtypes.bfloat16)


def make_in_maps(x, W1, W2, Wq, bq, Wk, bk, Wv, bv, Wo, bo, W3):
    """Host-side sharding: slicing / transposition / tiny-constant assembly."""
    f32 = np.float32
    x = np.asarray(x, f32)
    W1, W2, W3 = (np.asarray(a, f32) for a in (W1, W2, W3))
    Wq, Wk, Wv, Wo = (np.asarray(a, f32) for a in (Wq, Wk, Wv, Wo))
    w1e = np.concatenate([W1.T, np.ones((1, HID), f32)], axis=0)  # [4,16]
    w2e = np.concatenate([W2.T, np.ones((1, HID), f32)], axis=0)  # [5,16]
    m5t = (w2e @ w1e.T).T                                          # [4,5]
    w32 = W3 @ w2e.T                                               # [3,5]
    w32t = np.zeros((CH2 + 2, CH1 + 1), f32)                       # [6,4]
    w32t[0:5, 0:3] = w32.T
    w32t[5, CH1] = 1.0
    w3sr = np.tile(W3.sum(axis=1), B)                              # [192]
    bias3 = np.stack([np.tile(np.asarray(bq, f32), BLOC),
                      np.tile(np.asarray(bk, f32), BLOC),
                      np.tile(np.asarray(bv, f32), BLOC),
                      np.ones(BLOC * E, f32)], axis=0)              # [4,8E]
    shared = {"bias3": bf(bias3), "m5t": bf(m5t), "w32t": bf(w32t)}

    in_maps = []
    for c in range(NCORES):
        sl = slice(c * TLOC, (c + 1) * TLOC)
        xt_r = x[:, :, sl].transpose(2, 1, 0)          # [2048, 7, 64]
        xt_r = xt_r.reshape(KT, 128, XW).transpose(1, 0, 2).reshape(
            128, KT * XW)
        wq_r = Wq[:, sl].T.reshape(KT, 128, E).transpose(1, 0, 2).reshape(
            128, KT * E)
        wk_r = Wk[:, sl].T.reshape(KT, 128, E).transpose(1, 0, 2).reshape(
            128, KT * E)
        wv_r = Wv[:, sl].T.reshape(KT, 128, E).transpose(1, 0, 2).reshape(
            128, KT * E)
        wot_r = Wo[sl, :].T.reshape(EC, 128, TLOC).transpose(1, 0, 2).reshape(
            128, EC * TLOC)
        botw = np.concatenate([np.asarray(bo, f32)[sl], w3sr])[None, :]
        m = {"xt": bf(xt_r), "wq": bf(wq_r), "wk": bf(wk_r), "wv": bf(wv_r),
             "wot": bf(wot_r), "botw": bf(botw)}
        m.update(shared)
        in_maps.append(m)
    return in_maps


def assemble_output(results):
    """[per-core yt [128, KT*192]] -> [B, CH1, T]; t = tc*128 + p and
    col = tc*192 + g*24 + bl*3 + o."""
    arr = np.stack([np.asarray(res["yt"], np.float32) for res in results],
                   axis=0)                    # [rc, 128, KT*192]
    arr = arr.reshape(NCORES, 128, KT, NCORES, BLOC, CH1)
    # dims: (rc, p, tc, g, bl, o) -> b=(g,bl), c=o, T=(rc, tc, p)
    return np.ascontiguousarray(
        arr.transpose(3, 4, 5, 0, 2, 1).reshape(B, CH1, T))


def run(inputs, reps: int = 1):
    runner = _get_runner(reps)
    in_maps = make_in_maps(**inputs)
    results = runner(in_maps)
    return assemble_output(results)


def kernel(**inputs) -> np.ndarray:
    return run(inputs, reps=1)


def time_reps(inputs, reps: int, n: int = 10):
    """Per-call wall times with device-resident inputs (first call = warmup)."""
    import time
    import jax
    from jax.sharding import NamedSharding, PartitionSpec

    runner = _get_runner(reps)
    in_maps = make_in_maps(**inputs)
    concat = [
        np.concatenate([np.asarray(in_maps[c][nm]) for c in range(NCORES)],
                       axis=0)
        for nm in runner.in_names]
    sh = NamedSharding(runner.mesh, PartitionSpec("core"))
    dev = [jax.device_put(a, sh) for a in concat]
    times = []
    for i in range(n + 1):
        zeros = [np.zeros((NCORES * z.shape[0], *z.shape[1:]), z.dtype)
                 for z in runner.zero_outs]
        t0 = time.perf_counter()
        out = runner.fn(*dev, *zeros)
        jax.block_until_ready(out)
        dt = time.perf_counter() - t0
        if i > 0:
            times.append(dt)
    return times
